# revision 6
# baseline (speedup 1.0000x reference)
"""GQA attention block (RMSNorm-QK, causal, GQA) on 8 trn2 NeuronCores.

v10: strided-query sharding, bf16 end-to-end (bf16 output, host-widened),
causal skip, sharded K/V with AllGather, and the output projection's first
half-contraction interleaved into the attention pipeline.

Core c handles batch c//4 and query tokens {t : t % 4 == r}, r = c%4. Strided
queries make the causal structure identical on every core (token of local
query i is 4i+r), so one uniform SPMD program can *skip* fully-masked key
tiles: scores/den/context matmuls for key-tile kt only cover query columns
[128*(kt//4) : 512] — 5120 moving columns per head instead of 8192. The
128-token diagonal band is handled by 4 per-core [128,128] mask tiles
(host data), multiplied into the first 128 columns post-exp. K/V projection
is sharded: each core projects only its contiguous 512-token chunk, then an
AllGather over {0..3}/{4..7} assembles full K/V while Q projects in parallel
(the gather fits exactly in this DMA-quiet window). Heads 0-7 run a
paired-EXP attention pipeline (scalar would otherwise bind); heads 8-15 drop
pairing to free two PSUM banks so the ct=0..7 half of the output projection
streams into the attention PE bubbles (fillers emitted before the
dependency-stalled matmuls — the PE queue is in-order), halving the serial
out-proj tail; the recombine fuses (psum + bias) + partial in one
scalar_tensor_tensor.

All big matmuls are bf16 (host-converted weights/x): full PE rate, 4x faster
weight loads, half the DMA. Rank-1 broadcast matmuls (RMS/softmax denom
broadcasts) are f32r (fp32 runs at 1/4 rate). Softmax denominators use
reciprocal_approx_fast (5x faster than reciprocal; 18-bit accuracy is far
below the 2e-2 gate). V is projected directly in [token, feature] layout
(x-tile stationary, Wv moving) so no PE transposes exist anywhere. Wq/Wo
live whole in SBUF (8MB bf16 each), chunk-DMA'd during the preceding phase.
"""

import math
import numpy as np
from contextlib import ExitStack

import concourse.bass as bass
import concourse.mybir as mybir
import concourse.tile as tile
from concourse import bacc
from concourse.bass_utils import run_bass_kernel_spmd

F32 = mybir.dt.float32
F32R = mybir.dt.float32r
BF16 = mybir.dt.bfloat16
ADD = mybir.AluOpType.add
MULT = mybir.AluOpType.mult
EXP = mybir.ActivationFunctionType.Exp
SQRT = mybir.ActivationFunctionType.Sqrt
SQUARE = mybir.ActivationFunctionType.Square

EPS = 1e-8


def full_cfg():
    return dict(B=2, S=2048, E=2048, D=128, G=2)


def derived(cfg):
    B, S, E, D, G = cfg["B"], cfg["S"], cfg["E"], cfg["D"], cfg["G"]
    NH = E // D            # 16 query heads
    ET = E // 128          # 16 feature tiles
    NKT = S // 128         # 16 key tiles
    QPC = S // 4           # 512 queries per core
    GS = NH // G           # 8 heads per kv group
    assert D == 128 and QPC == 512
    return NH, ET, NKT, QPC, GS


def build_program(cfg):
    B, S, E, D, G = cfg["B"], cfg["S"], cfg["E"], cfg["D"], cfg["G"]
    NH, ET, NKT, QPC, GS = derived(cfg)
    SCALE = 1.0 / math.sqrt(D)
    KC = 512               # token-chunk width for K/V projection
    NKC = S // KC
    GD = G * D             # 256

    nc = bacc.Bacc()
    xkv_d = nc.dram_tensor("xkv", [E, 512], BF16, kind="ExternalInput")
    xq_d = nc.dram_tensor("xq", [E, QPC], BF16, kind="ExternalInput")
    wq_d = nc.dram_tensor("Wq", [E, E], BF16, kind="ExternalInput")
    wk_d = nc.dram_tensor("Wk", [E, GD], BF16, kind="ExternalInput")
    wv_d = nc.dram_tensor("Wv", [E, GD], BF16, kind="ExternalInput")
    wo_d = nc.dram_tensor("Wo", [E, E], BF16, kind="ExternalInput")
    bq_d = nc.dram_tensor("bq_t", [128, ET], F32, kind="ExternalInput")
    bk_d = nc.dram_tensor("bk_t", [128, G], F32, kind="ExternalInput")
    bv_d = nc.dram_tensor("bv_r", [1, GD], BF16, kind="ExternalInput")
    bo_d = nc.dram_tensor("bo_t", [128, ET], F32, kind="ExternalInput")
    gq_d = nc.dram_tensor("gq_c", [128, 1], F32, kind="ExternalInput")
    gk_d = nc.dram_tensor("gk_c", [128, 1], F32, kind="ExternalInput")
    mask_d = nc.dram_tensor("mask", [4 * 128, 128], BF16, kind="ExternalInput")
    onesq_d = nc.dram_tensor("ones_sq", [128, 128], F32, kind="ExternalInput")
    out_d = nc.dram_tensor("outT", [E, QPC], BF16, kind="ExternalOutput")

    wq_r = wq_d.rearrange("(t p) c -> p t c", p=128)   # [128, ET, E]
    wk_r = wk_d.rearrange("(t p) c -> p t c", p=128)   # [128, ET, GD]
    wv_r = wv_d.rearrange("(t p) c -> p t c", p=128)
    wo_r = wo_d.rearrange("(t p) c -> p t c", p=128)

    def r(ap):
        return ap if ap.dtype == F32R else ap.bitcast(F32R)

    with tile.TileContext(nc) as tc, ExitStack() as top:
        consts = top.enter_context(tc.tile_pool(name="consts", bufs=1))
        persist = top.enter_context(tc.tile_pool(name="persist", bufs=1))

        ones_sq_r = consts.tile([128, 128], F32R)
        ones_sq_bf = consts.tile([128, 128], BF16)
        nc.vector.memset(ones_sq_bf, 1.0)
        onesrow_bf = consts.tile([1, 128], BF16)
        nc.vector.memset(onesrow_bf, 1.0)
        eps_c = consts.tile([128, 1], F32)
        nc.vector.memset(eps_c, EPS)
        gq_sb = consts.tile([128, 1], F32)
        gk_sb = consts.tile([128, 1], F32)
        bq_sb = consts.tile([128, ET], F32)
        bk_sb = consts.tile([128, G], F32)
        bv_sb = consts.tile([1, GD], BF16)
        bo_sb = consts.tile([128, ET], F32)
        mask_sb = [consts.tile([128, 128], BF16, tag=f"mask{m}",
                               name=f"mask{m}") for m in range(4)]

        def issue_const_dmas():
            nc.sync.dma_start(out=ones_sq_r, in_=onesq_d[:, :].bitcast(F32R))
            nc.sync.dma_start(out=gq_sb, in_=gq_d[:, :])
            nc.sync.dma_start(out=gk_sb, in_=gk_d[:, :])
            nc.sync.dma_start(out=bq_sb, in_=bq_d[:, :])
            nc.sync.dma_start(out=bk_sb, in_=bk_d[:, :])
            nc.sync.dma_start(out=bv_sb, in_=bv_d[:, :])
            nc.sync.dma_start(out=bo_sb, in_=bo_d[:, :])
            for m in range(4):
                nc.sync.dma_start(out=mask_sb[m],
                                  in_=mask_d[m * 128:(m + 1) * 128, :])

        ktn = [persist.tile([128, S], BF16, tag=f"ktn{g}", name=f"ktn{g}")
               for g in range(G)]
        vtok = persist.tile([128, NKT, GD], BF16, tag="vtok")
        qtn = persist.tile([128, NH, QPC], BF16, tag="qtn")

        # ---------------- phase 1: K/V projections ----------------------
        # wq_all/xq_all persist into phase 2 (freed before Wo loads); DMAs
        # are issued mid-phase-1 so the transfers hide under K/V compute.
        pwq = ExitStack()
        wqxq = pwq.enter_context(tc.tile_pool(name="wqxq", bufs=1))
        wq_all = wqxq.tile([128, ET, E], BF16, tag="wq_all")
        xq_all = wqxq.tile([128, ET, QPC], BF16, tag="xq_all")

        with ExitStack() as p1:
            wkvp = p1.enter_context(tc.tile_pool(name="wkv", bufs=1))
            xsp = p1.enter_context(tc.tile_pool(name="xs", bufs=17))
            stg = p1.enter_context(tc.tile_pool(name="stg", bufs=1))
            dramp = p1.enter_context(tc.tile_pool(name="ccd", bufs=1,
                                                  space="DRAM"))
            tmp = p1.enter_context(tc.tile_pool(name="tmp1", bufs=3))
            pkv = p1.enter_context(tc.tile_pool(name="pkv", bufs=2, space="PSUM"))
            pv = p1.enter_context(tc.tile_pool(name="pv", bufs=2, space="PSUM"))
            pssq = p1.enter_context(tc.tile_pool(name="pssq", bufs=2, space="PSUM"))

            wk_sb = wkvp.tile([128, ET, GD], BF16, tag="wk")
            wv_sb = wkvp.tile([128, ET, GD], BF16, tag="wv")
            kst = stg.tile([128, G, 512], BF16, tag="kst")
            vst = stg.tile([128, 4, GD], BF16, tag="vst")
            cc_in = dramp.tile([128, 2048], BF16, tag="cc_in")
            cc_out = dramp.tile([4, 128, 2048], BF16, tag="cc_out")

            xts = []
            for et in range(ET):
                xt = xsp.tile([128, KC], BF16, tag="xt")
                nc.sync.dma_start(
                    out=xt, in_=xkv_d[et * 128:(et + 1) * 128, :])
                xts.append(xt)
                # interleave weight chunks across DMA queues so the first
                # matmul isn't gated on a single 1MB transfer
                nc.sync.dma_start(out=wk_sb[:, et, :], in_=wk_r[:, et, :])
                nc.sync.dma_start(out=wv_sb[:, et, :], in_=wv_r[:, et, :])
            issue_const_dmas()
            for et in range(ET):
                nc.sync.dma_start(out=wq_all[:, et, :], in_=wq_r[:, et, :])
                nc.sync.dma_start(out=xq_all[:, et, :],
                                  in_=xq_d[et * 128:(et + 1) * 128, :])

            pending = []

            def flush():
                while pending:
                    pending.pop(0)()

            # K projection for this core's 512-token chunk: [d, token]
            for g in range(G):
                acc = pkv.tile([128, KC], F32, tag="pkv", name="kacc")
                for et in range(ET):
                    nc.tensor.matmul(
                        acc, lhsT=wk_sb[:, et, g * D:(g + 1) * D],
                        rhs=xts[et], start=(et == 0), stop=(et == ET - 1))

                def post_k(g=g, acc=acc):
                    vb = tmp.tile([128, KC], F32, tag="vb", name="kb")
                    nc.vector.tensor_scalar(
                        out=vb, in0=acc, scalar1=bk_sb[:, g:g + 1],
                        scalar2=None, op0=ADD)
                    sq = tmp.tile([128, KC], F32R, tag="sq", name="ksq")
                    nc.scalar.activation(out=sq, in_=vb, func=SQUARE)
                    # sum over partitions, pre-broadcast to all 128 rows
                    ssqb = pssq.tile([128, KC], F32, tag="ssq", name="kssq")
                    nc.tensor.matmul(ssqb, lhsT=ones_sq_r, rhs=sq,
                                     start=True, stop=True)
                    rmsb = tmp.tile([128, KC], F32, tag="rms", name="krms")
                    nc.scalar.activation(out=rmsb, in_=ssqb, func=SQRT,
                                         scale=1.0 / D, bias=eps_c[:, :])
                    rinvb = tmp.tile([128, KC], F32, tag="rinv",
                                     name="krinv")
                    nc.vector.reciprocal_approx_fast(out=rinvb, in_=rmsb)
                    nc.vector.scalar_tensor_tensor(
                        out=kst[:, g, :], in0=vb, scalar=gk_sb[:, 0:1],
                        in1=rinvb, op0=MULT, op1=MULT)
                pending.append(post_k)

            # V projection directly as [token, feature(GD)]
            for s2 in range(2):
                vt2 = pv.tile([128, 2, GD], F32, tag="pv", name="vt2")
                for s in range(2):
                    sub = 2 * s2 + s
                    for et in range(ET):
                        nc.tensor.matmul(
                            vt2[:, s, :],
                            lhsT=xts[et][:, sub * 128:(sub + 1) * 128],
                            rhs=wv_sb[:, et, :],
                            start=(et == 0), stop=False)
                    nc.tensor.matmul(
                        vt2[:, s, :], lhsT=onesrow_bf, rhs=bv_sb,
                        start=False, stop=True)

                def post_v(s2=s2, vt2=vt2):
                    nc.scalar.copy(out=vst[:, s2 * 2:s2 * 2 + 2, :], in_=vt2)
                pending.append(post_v)
                if len(pending) > 2:
                    pending.pop(0)()
            flush()

            # gather K/V chunks from the 3 peer cores of this batch
            nc.gpsimd.dma_start(out=cc_in[:, 0:1024], in_=kst[:, :, :])
            nc.gpsimd.dma_start(out=cc_in[:, 1024:2048], in_=vst[:, :, :])
            nc.gpsimd.collective_compute(
                "AllGather", mybir.AluOpType.bypass,
                replica_groups=[[0, 1, 2, 3], [4, 5, 6, 7]],
                ins=[cc_in.opt()], outs=[cc_out.opt()])
            for rr in range(4):
                for g in range(G):
                    nc.sync.dma_start(
                        out=ktn[g][:, rr * 512:(rr + 1) * 512],
                        in_=cc_out[rr, :, g * 512:(g + 1) * 512])
                nc.sync.dma_start(
                    out=vtok[:, rr * 4:(rr + 1) * 4, :],
                    in_=cc_out[rr, :, 1024:2048])

        # ---------------- phase 2: Q projection -------------------------
        with ExitStack() as p2:
            tmp2 = p2.enter_context(tc.tile_pool(name="tmp2", bufs=3))
            pq = p2.enter_context(tc.tile_pool(name="pq", bufs=2, space="PSUM"))
            pssq2 = p2.enter_context(tc.tile_pool(name="pssq2", bufs=2, space="PSUM"))
            pending = []
            for qc in range(NH):
                acc = pq.tile([128, QPC], F32, tag="pq", name="qacc")
                for et in range(ET):
                    nc.tensor.matmul(
                        acc, lhsT=wq_all[:, et, qc * 128:(qc + 1) * 128],
                        rhs=xq_all[:, et, :],
                        start=(et == 0), stop=(et == ET - 1))

                def post_q(qc=qc, acc=acc):
                    vb = tmp2.tile([128, QPC], F32, tag="vb", name="qb")
                    nc.vector.tensor_scalar(
                        out=vb, in0=acc, scalar1=bq_sb[:, qc:qc + 1],
                        scalar2=None, op0=ADD)
                    sq = tmp2.tile([128, QPC], F32R, tag="sq", name="qsq")
                    nc.scalar.activation(out=sq, in_=vb, func=SQUARE)
                    ssqb = pssq2.tile([128, QPC], F32, tag="ssq", name="qssq")
                    nc.tensor.matmul(ssqb, lhsT=ones_sq_r, rhs=sq,
                                     start=True, stop=True)
                    rmsb = tmp2.tile([128, QPC], F32, tag="rms", name="qrms")
                    nc.scalar.activation(out=rmsb, in_=ssqb, func=SQRT,
                                         scale=1.0 / D, bias=eps_c[:, :])
                    rinvb = tmp2.tile([128, QPC], F32, tag="rinv",
                                      name="qrinv")
                    nc.vector.reciprocal_approx_fast(out=rinvb, in_=rmsb)
                    nc.vector.scalar_tensor_tensor(
                        out=qtn[:, qc, :], in0=vb, scalar=gq_sb[:, 0:1],
                        in1=rinvb, op0=MULT, op1=MULT)
                pending.append(post_q)
                if len(pending) > 1:
                    pending.pop(0)()
            while pending:
                pending.pop(0)()
        pwq.close()

        # ---------------- phase 3: attention + out proj -----------------
        with ExitStack() as p34:
            ctxp = p34.enter_context(tc.tile_pool(name="ctxp", bufs=1))
            ctxt = ctxp.tile([128, NH, QPC], BF16, tag="ctxt", name="ctxt")
            wop = p34.enter_context(tc.tile_pool(name="wos", bufs=1))
            wo_all = wop.tile([128, ET, E], BF16, tag="wo_all")
            for et in range(ET):
                nc.sync.dma_start(out=wo_all[:, et, :], in_=wo_r[:, et, :])
            ptp = p34.enter_context(tc.tile_pool(name="pt", bufs=3))
            hdp = p34.enter_context(tc.tile_pool(name="hdp", bufs=2))
            osb = p34.enter_context(tc.tile_pool(name="osb", bufs=2))
            oap = p34.enter_context(tc.tile_pool(name="oacc", bufs=1))
            outacc = oap.tile([128, ET, QPC], F32, tag="outacc")
            pcx = p34.enter_context(tc.tile_pool(name="pcx", bufs=2, space="PSUM"))
            pden = p34.enter_context(tc.tile_pool(name="pden", bufs=2, space="PSUM"))
            pending2 = []

            def flush2():
                while pending2:
                    pending2.pop(0)()

            def head_common(h, den, cx):
                def post_head(h=h, den=den, cx=cx):
                    rdb = hdp.tile([128, QPC], F32, tag="rd", name="rd")
                    nc.vector.reciprocal_approx_fast(out=rdb, in_=den)
                    nc.vector.tensor_tensor(out=ctxt[:, h, :], in0=cx,
                                            in1=rdb, op=MULT)
                pending2.append(post_head)

            # heads 0-7: paired-EXP pipeline (scalar would otherwise bind)
            with ExitStack() as pA:
                psc = pA.enter_context(tc.tile_pool(name="pscp", bufs=2,
                                                    space="PSUM"))
                for h in range(GS):
                    g = h // GS
                    den = pden.tile([128, QPC], F32, tag="den", name="den")
                    cx = pcx.tile([128, QPC], F32, tag="cx", name="cx")
                    for gi in range(4):
                        W = QPC - 128 * gi
                        q_ap = qtn[:, h, 128 * gi:QPC]
                        for mp in range(2):
                            sc2 = psc.tile([128, 2, QPC], F32, tag="sc",
                                           name="sc")
                            for j in range(2):
                                kt = 4 * gi + 2 * mp + j
                                nc.tensor.matmul(
                                    sc2[:, j, 0:W],
                                    lhsT=ktn[g][:, kt * 128:(kt + 1) * 128],
                                    rhs=q_ap, start=True, stop=True)

                            def post_sc(h=h, g=g, gi=gi, mp=mp, W=W,
                                        sc2=sc2, den=den, cx=cx):
                                pt2 = ptp.tile([128, 2, QPC], BF16,
                                               tag="pt", name="pt")
                                nc.scalar.activation(
                                    out=pt2[:, :, 0:W], in_=sc2[:, :, 0:W],
                                    func=EXP, scale=SCALE)
                                for j in range(2):
                                    kt = 4 * gi + 2 * mp + j
                                    nc.vector.tensor_tensor(
                                        out=pt2[:, j, 0:128],
                                        in0=pt2[:, j, 0:128],
                                        in1=mask_sb[2 * mp + j], op=MULT)
                                    nc.tensor.matmul(
                                        den[:, 128 * gi:QPC],
                                        lhsT=ones_sq_bf,
                                        rhs=pt2[:, j, 0:W],
                                        start=(kt == 0),
                                        stop=(kt == NKT - 1),
                                        skip_group_check=True)
                                    nc.tensor.matmul(
                                        cx[:, 128 * gi:QPC],
                                        lhsT=vtok[:, kt, g * D:(g + 1) * D],
                                        rhs=pt2[:, j, 0:W],
                                        start=(kt == 0),
                                        stop=(kt == NKT - 1),
                                        skip_group_check=True)
                            pending2.append(post_sc)
                            if len(pending2) > 2:
                                pending2.pop(0)()
                    head_common(h, den, cx)
                flush2()

            # heads 8-15: single-EXP pipeline, freeing two PSUM banks so the
            # ct=0..7 half of the output projection interleaves into the PE
            # bubbles (ctxt of heads 0-7 is final by now)
            with ExitStack() as pB:
                psc = pB.enter_context(tc.tile_pool(name="pscs", bufs=2,
                                                    space="PSUM"))
                pout = pB.enter_context(tc.tile_pool(name="pout", bufs=2,
                                                     space="PSUM"))
                ost = {"i": 0, "acc": None}

                def emit_out_slot():
                    i = ost["i"]
                    if i >= GS * ET:
                        return
                    ost["i"] += 1
                    c2, ct = i // GS, i % GS
                    if ct == 0:
                        ost["acc"] = pout.tile([128, QPC], F32, tag="po",
                                               name="po")
                    acc = ost["acc"]
                    nc.tensor.matmul(
                        acc, lhsT=wo_all[:, ct, c2 * 128:(c2 + 1) * 128],
                        rhs=ctxt[:, ct, :],
                        start=(ct == 0), stop=(ct == GS - 1))
                    if ct == GS - 1:
                        nc.scalar.copy(out=outacc[:, c2, :], in_=acc)

                for h in range(GS, NH):
                    g = h // GS
                    den = pden.tile([128, QPC], F32, tag="den", name="den")
                    cx = pcx.tile([128, QPC], F32, tag="cx", name="cx")
                    for gi in range(4):
                        W = QPC - 128 * gi
                        q_ap = qtn[:, h, 128 * gi:QPC]
                        for m in range(4):
                            kt = 4 * gi + m
                            sc = psc.tile([128, QPC], F32, tag="sc",
                                          name="sc")
                            nc.tensor.matmul(
                                sc[:, 0:W],
                                lhsT=ktn[g][:, kt * 128:(kt + 1) * 128],
                                rhs=q_ap, start=True, stop=True)

                            def post_sc(h=h, g=g, gi=gi, m=m, kt=kt, W=W,
                                        sc=sc, den=den, cx=cx):
                                pt = ptp.tile([128, QPC], BF16, tag="pt1",
                                              name="pt1")
                                nc.scalar.activation(
                                    out=pt[:, 0:W], in_=sc[:, 0:W],
                                    func=EXP, scale=SCALE)
                                nc.vector.tensor_tensor(
                                    out=pt[:, 0:128], in0=pt[:, 0:128],
                                    in1=mask_sb[m], op=MULT)
                                emit_out_slot()
                                nc.tensor.matmul(
                                    den[:, 128 * gi:QPC], lhsT=ones_sq_bf,
                                    rhs=pt[:, 0:W], start=(kt == 0),
                                    stop=(kt == NKT - 1),
                                    skip_group_check=True)
                                nc.tensor.matmul(
                                    cx[:, 128 * gi:QPC],
                                    lhsT=vtok[:, kt, g * D:(g + 1) * D],
                                    rhs=pt[:, 0:W], start=(kt == 0),
                                    stop=(kt == NKT - 1),
                                    skip_group_check=True)
                            pending2.append(post_sc)
                            if len(pending2) > 3:
                                pending2.pop(0)()
                    head_common(h, den, cx)
                flush2()

                # remaining half-contraction (ct=8..15) + fused combine
                for c2 in range(ET):
                    acc = pout.tile([128, QPC], F32, tag="po", name="po2")
                    for ct in range(GS, ET):
                        nc.tensor.matmul(
                            acc,
                            lhsT=wo_all[:, ct, c2 * 128:(c2 + 1) * 128],
                            rhs=ctxt[:, ct, :],
                            start=(ct == GS), stop=(ct == ET - 1))

                    def post_o(c2=c2, acc=acc):
                        ot = osb.tile([128, QPC], BF16, tag="ot", name="ot")
                        nc.vector.scalar_tensor_tensor(
                            out=ot, in0=acc, scalar=bo_sb[:, c2:c2 + 1],
                            in1=outacc[:, c2, :], op0=ADD, op1=ADD)
                        nc.sync.dma_start(
                            out=out_d[c2 * 128:(c2 + 1) * 128, :], in_=ot)
                    pending2.append(post_o)
                    if len(pending2) > 1:
                        pending2.pop(0)()
                flush2()
    nc.compile()
    return nc


# ---------------------------------------------------------------------------
# host-side sharding
# ---------------------------------------------------------------------------

def core_masks(cfg, rr):
    """[4*128, 128] bf16 diag-band masks: keep iff 128m + k <= 4q + r."""
    import ml_dtypes
    m = np.zeros((4 * 128, 128), np.float32)
    kk = np.arange(128)[:, None]
    qq = np.arange(128)[None, :]
    for t in range(4):
        m[t * 128:(t + 1) * 128, :] = (128 * t + kk <= 4 * qq + rr)
    return m.astype(ml_dtypes.bfloat16)


def make_in_maps(cfg, inputs):
    import ml_dtypes
    BF = ml_dtypes.bfloat16
    B, S, E, D, G = cfg["B"], cfg["S"], cfg["E"], cfg["D"], cfg["G"]
    NH, ET, NKT, QPC, GS = derived(cfg)
    x = np.asarray(inputs["x"], np.float32)
    shared = dict(
        Wq=np.ascontiguousarray(np.asarray(inputs["Wq"], np.float32)).astype(BF),
        Wk=np.ascontiguousarray(np.asarray(inputs["Wk"], np.float32)).astype(BF),
        Wv=np.ascontiguousarray(np.asarray(inputs["Wv"], np.float32)).astype(BF),
        Wo=np.ascontiguousarray(np.asarray(inputs["Wo"], np.float32)).astype(BF),
        bq_t=np.ascontiguousarray(
            np.asarray(inputs["bq"], np.float32).reshape(ET, 128).T),
        bk_t=np.ascontiguousarray(
            np.asarray(inputs["bk"], np.float32).reshape(G, 128).T),
        bv_r=np.asarray(inputs["bv"], np.float32).reshape(1, G * D).astype(BF),
        bo_t=np.ascontiguousarray(
            np.asarray(inputs["bo"], np.float32).reshape(ET, 128).T),
        gq_c=np.ascontiguousarray(
            np.asarray(inputs["gamma_q"], np.float32).reshape(128, 1)),
        gk_c=np.ascontiguousarray(
            np.asarray(inputs["gamma_k"], np.float32).reshape(128, 1)),
        ones_sq=np.ones((128, 128), np.float32),
    )
    xTb = [np.ascontiguousarray(x[b].T).astype(BF) for b in range(B)]
    in_maps = []
    for c in range(8):
        b, rr = c // 4, c % 4
        m = dict(shared)
        m["xkv"] = np.ascontiguousarray(xTb[b][:, rr * 512:(rr + 1) * 512])
        m["xq"] = np.ascontiguousarray(xTb[b][:, rr::4])
        m["mask"] = core_masks(cfg, rr)
        in_maps.append(m)
    return in_maps


def assemble(cfg, results):
    B, S, E = cfg["B"], cfg["S"], cfg["E"]
    out = np.empty((B, S, E), np.float32)
    for c in range(8):
        b, rr = c // 4, c % 4
        out[b, rr::4, :] = results[c]["outT"].T.astype(np.float32)
    return out


_CACHE = {}


def kernel(**inputs):
    cfg = full_cfg()
    if "nc" not in _CACHE:
        _CACHE["nc"] = build_program(cfg)
    nc = _CACHE["nc"]
    in_maps = make_in_maps(cfg, inputs)
    res = run_bass_kernel_spmd(nc, in_maps, list(range(8)))
    return assemble(cfg, res.results)


# revision 7
# speedup vs baseline: 1.0202x; 1.0202x over previous
"""GQA attention block (RMSNorm-QK, causal, GQA) on 8 trn2 NeuronCores.

v12: strided-query sharding, bf16 end-to-end (bf16 output, host-widened),
causal skip, zero collectives, and the output projection's first
half-contraction interleaved into the attention pipeline.

Core c handles batch c//4 and query tokens {t : t % 4 == r}, r = c%4. Strided
queries make the causal structure identical on every core (token of local
query i is 4i+r), so one uniform SPMD program can *skip* fully-masked key
tiles: scores/den/context matmuls for key-tile kt only cover query columns
[128*(kt//4) : 512] — 5120 moving columns per head instead of 8192. The
128-token diagonal band is handled by 4 per-core [128,128] mask tiles
(host data), multiplied into the first 128 columns post-exp. Every core
projects K/V for its full batch locally (a 4-way-sharded AllGather variant
measured the same mean latency with far higher variance; the gather's
contention-dependent completion gated attention start). Heads 0-7 run a
paired-EXP attention pipeline (scalar would otherwise bind); heads 8-15
drop pairing to free two PSUM banks so the ct=0..7 half of the output
projection streams into the attention PE bubbles (fillers emitted before
the dependency-stalled matmuls — the PE queue is in-order), halving the
serial out-proj tail; the recombine fuses (psum + bias) + partial in one
scalar_tensor_tensor.

All big matmuls are bf16 (host-converted weights/x): full PE rate, 4x faster
weight loads, half the DMA. Rank-1 broadcast matmuls (RMS/softmax denom
broadcasts) are f32r (fp32 runs at 1/4 rate). Softmax denominators use
reciprocal_approx_fast (5x faster than reciprocal; 18-bit accuracy is far
below the 2e-2 gate). V is projected directly in [token, feature] layout
(x-tile stationary, Wv moving) so no PE transposes exist anywhere. Wq/Wo
live whole in SBUF (8MB bf16 each), chunk-DMA'd during the preceding phase.
"""

import math
import numpy as np
from contextlib import ExitStack

import concourse.bass as bass
import concourse.mybir as mybir
import concourse.tile as tile
from concourse import bacc
from concourse.bass_utils import run_bass_kernel_spmd

F32 = mybir.dt.float32
F32R = mybir.dt.float32r
BF16 = mybir.dt.bfloat16
ADD = mybir.AluOpType.add
MULT = mybir.AluOpType.mult
EXP = mybir.ActivationFunctionType.Exp
SQRT = mybir.ActivationFunctionType.Sqrt
SQUARE = mybir.ActivationFunctionType.Square

EPS = 1e-8


def full_cfg():
    return dict(B=2, S=2048, E=2048, D=128, G=2)


def derived(cfg):
    B, S, E, D, G = cfg["B"], cfg["S"], cfg["E"], cfg["D"], cfg["G"]
    NH = E // D            # 16 query heads
    ET = E // 128          # 16 feature tiles
    NKT = S // 128         # 16 key tiles
    QPC = S // 4           # 512 queries per core
    GS = NH // G           # 8 heads per kv group
    assert D == 128 and QPC == 512
    return NH, ET, NKT, QPC, GS


def build_program(cfg):
    B, S, E, D, G = cfg["B"], cfg["S"], cfg["E"], cfg["D"], cfg["G"]
    NH, ET, NKT, QPC, GS = derived(cfg)
    SCALE = 1.0 / math.sqrt(D)
    KC = 512               # token-chunk width for K/V projection
    NKC = S // KC
    GD = G * D             # 256

    nc = bacc.Bacc()
    xT_d = nc.dram_tensor("xT", [E, S], BF16, kind="ExternalInput")
    xq_d = nc.dram_tensor("xq", [E, QPC], BF16, kind="ExternalInput")
    wq_d = nc.dram_tensor("Wq", [E, E], BF16, kind="ExternalInput")
    wk_d = nc.dram_tensor("Wk", [E, GD], BF16, kind="ExternalInput")
    wv_d = nc.dram_tensor("Wv", [E, GD], BF16, kind="ExternalInput")
    wo_d = nc.dram_tensor("Wo", [E, E], BF16, kind="ExternalInput")
    bq_d = nc.dram_tensor("bq_t", [128, ET], F32, kind="ExternalInput")
    bk_d = nc.dram_tensor("bk_t", [128, G], F32, kind="ExternalInput")
    bv_d = nc.dram_tensor("bv_r", [1, GD], BF16, kind="ExternalInput")
    bo_d = nc.dram_tensor("bo_t", [128, ET], F32, kind="ExternalInput")
    gq_d = nc.dram_tensor("gq_c", [128, 1], F32, kind="ExternalInput")
    gk_d = nc.dram_tensor("gk_c", [128, 1], F32, kind="ExternalInput")
    mask_d = nc.dram_tensor("mask", [4 * 128, 128], BF16, kind="ExternalInput")
    onesq_d = nc.dram_tensor("ones_sq", [128, 128], F32, kind="ExternalInput")
    out_d = nc.dram_tensor("outT", [E, QPC], BF16, kind="ExternalOutput")

    wq_r = wq_d.rearrange("(t p) c -> p t c", p=128)   # [128, ET, E]
    wk_r = wk_d.rearrange("(t p) c -> p t c", p=128)   # [128, ET, GD]
    wv_r = wv_d.rearrange("(t p) c -> p t c", p=128)
    wo_r = wo_d.rearrange("(t p) c -> p t c", p=128)

    def r(ap):
        return ap if ap.dtype == F32R else ap.bitcast(F32R)

    with tile.TileContext(nc) as tc, ExitStack() as top:
        consts = top.enter_context(tc.tile_pool(name="consts", bufs=1))
        persist = top.enter_context(tc.tile_pool(name="persist", bufs=1))

        ones_sq_r = consts.tile([128, 128], F32R)
        ones_sq_bf = consts.tile([128, 128], BF16)
        nc.vector.memset(ones_sq_bf, 1.0)
        onesrow_bf = consts.tile([1, 128], BF16)
        nc.vector.memset(onesrow_bf, 1.0)
        eps_c = consts.tile([128, 1], F32)
        nc.vector.memset(eps_c, EPS)
        gq_sb = consts.tile([128, 1], F32)
        gk_sb = consts.tile([128, 1], F32)
        bq_sb = consts.tile([128, ET], F32)
        bk_sb = consts.tile([128, G], F32)
        bv_sb = consts.tile([1, GD], BF16)
        bo_sb = consts.tile([128, ET], F32)
        mask_sb = [consts.tile([128, 128], BF16, tag=f"mask{m}",
                               name=f"mask{m}") for m in range(4)]

        def issue_const_dmas():
            nc.sync.dma_start(out=ones_sq_r, in_=onesq_d[:, :].bitcast(F32R))
            nc.sync.dma_start(out=gq_sb, in_=gq_d[:, :])
            nc.sync.dma_start(out=gk_sb, in_=gk_d[:, :])
            nc.sync.dma_start(out=bq_sb, in_=bq_d[:, :])
            nc.sync.dma_start(out=bk_sb, in_=bk_d[:, :])
            nc.sync.dma_start(out=bv_sb, in_=bv_d[:, :])
            nc.sync.dma_start(out=bo_sb, in_=bo_d[:, :])
            for m in range(4):
                nc.sync.dma_start(out=mask_sb[m],
                                  in_=mask_d[m * 128:(m + 1) * 128, :])

        ktn = [persist.tile([128, S], BF16, tag=f"ktn{g}", name=f"ktn{g}")
               for g in range(G)]
        vtok = persist.tile([128, NKT, GD], BF16, tag="vtok")
        qtn = persist.tile([128, NH, QPC], BF16, tag="qtn")

        # ---------------- phase 1: K/V projections ----------------------
        # wq_all/xq_all persist into phase 2 (freed before Wo loads); DMAs
        # are issued mid-phase-1 so the transfers hide under K/V compute.
        pwq = ExitStack()
        wqxq = pwq.enter_context(tc.tile_pool(name="wqxq", bufs=1))
        wq_all = wqxq.tile([128, ET, E], BF16, tag="wq_all")
        xq_all = wqxq.tile([128, ET, QPC], BF16, tag="xq_all")

        with ExitStack() as p1:
            wkvp = p1.enter_context(tc.tile_pool(name="wkv", bufs=1))
            xsp = p1.enter_context(tc.tile_pool(name="xs", bufs=20))
            tmp = p1.enter_context(tc.tile_pool(name="tmp1", bufs=3))
            pkv = p1.enter_context(tc.tile_pool(name="pkv", bufs=3, space="PSUM"))
            pv = p1.enter_context(tc.tile_pool(name="pv", bufs=2, space="PSUM"))
            pssq = p1.enter_context(tc.tile_pool(name="pssq", bufs=2, space="PSUM"))

            wk_sb = wkvp.tile([128, ET, GD], BF16, tag="wk")
            wv_sb = wkvp.tile([128, ET, GD], BF16, tag="wv")

            pending = []

            def flush():
                while pending:
                    pending.pop(0)()

            for kc in range(NKC):
                xts = []
                for et in range(ET):
                    xt = xsp.tile([128, KC], BF16, tag="xt")
                    nc.sync.dma_start(
                        out=xt, in_=xT_d[et * 128:(et + 1) * 128,
                                         kc * KC:(kc + 1) * KC])
                    xts.append(xt)
                    if kc == 0:
                        # interleave weight chunks across DMA queues so the
                        # first matmul isn't gated on a single 1MB transfer
                        nc.sync.dma_start(out=wk_sb[:, et, :],
                                          in_=wk_r[:, et, :])
                        nc.sync.dma_start(out=wv_sb[:, et, :],
                                          in_=wv_r[:, et, :])
                if kc == 0:
                    issue_const_dmas()
                if kc == 1:
                    for et in range(ET):
                        nc.sync.dma_start(out=wq_all[:, et, :],
                                          in_=wq_r[:, et, :])
                if kc == 2:
                    for et in range(ET):
                        nc.sync.dma_start(
                            out=xq_all[:, et, :],
                            in_=xq_d[et * 128:(et + 1) * 128, :])

                # K projection: feature-major [d, token]
                for g in range(G):
                    acc = pkv.tile([128, KC], F32, tag="pkv", name="kacc")
                    for et in range(ET):
                        nc.tensor.matmul(
                            acc, lhsT=wk_sb[:, et, g * D:(g + 1) * D],
                            rhs=xts[et], start=(et == 0), stop=(et == ET - 1))

                    def post_k(kc=kc, g=g, acc=acc):
                        vb = tmp.tile([128, KC], F32, tag="vb", name="kb")
                        nc.vector.tensor_scalar(
                            out=vb, in0=acc, scalar1=bk_sb[:, g:g + 1],
                            scalar2=None, op0=ADD)
                        sq = tmp.tile([128, KC], F32R, tag="sq", name="ksq")
                        nc.scalar.activation(out=sq, in_=vb, func=SQUARE)
                        # sum over partitions, pre-broadcast to all 128 rows
                        ssqb = pssq.tile([128, KC], F32, tag="ssq",
                                         name="kssq")
                        nc.tensor.matmul(ssqb, lhsT=ones_sq_r, rhs=sq,
                                         start=True, stop=True)
                        rmsb = tmp.tile([128, KC], F32, tag="rms",
                                        name="krms")
                        nc.scalar.activation(out=rmsb, in_=ssqb, func=SQRT,
                                             scale=1.0 / D, bias=eps_c[:, :])
                        rinvb = tmp.tile([128, KC], F32, tag="rinv",
                                         name="krinv")
                        nc.vector.reciprocal_approx_fast(out=rinvb, in_=rmsb)
                        nc.vector.scalar_tensor_tensor(
                            out=ktn[g][:, kc * KC:(kc + 1) * KC],
                            in0=vb, scalar=gk_sb[:, 0:1], in1=rinvb,
                            op0=MULT, op1=MULT)
                    pending.append(post_k)

                # V projection directly as [token, feature(GD)]
                for s2 in range(2):
                    vt2 = pv.tile([128, 2, GD], F32, tag="pv", name="vt2")
                    for s in range(2):
                        sub = 2 * s2 + s
                        for et in range(ET):
                            nc.tensor.matmul(
                                vt2[:, s, :],
                                lhsT=xts[et][:, sub * 128:(sub + 1) * 128],
                                rhs=wv_sb[:, et, :],
                                start=(et == 0), stop=False)
                        nc.tensor.matmul(
                            vt2[:, s, :], lhsT=onesrow_bf, rhs=bv_sb,
                            start=False, stop=True)

                    def post_v(kc=kc, s2=s2, vt2=vt2):
                        kt0 = kc * 4 + s2 * 2
                        nc.scalar.copy(out=vtok[:, kt0:kt0 + 2, :], in_=vt2)
                    pending.append(post_v)
                    if len(pending) > 2:
                        pending.pop(0)()
            flush()

        # ---------------- phase 2: Q projection -------------------------
        with ExitStack() as p2:
            tmp2 = p2.enter_context(tc.tile_pool(name="tmp2", bufs=3))
            pq = p2.enter_context(tc.tile_pool(name="pq", bufs=2, space="PSUM"))
            pssq2 = p2.enter_context(tc.tile_pool(name="pssq2", bufs=2, space="PSUM"))
            pending = []
            for qc in range(NH):
                acc = pq.tile([128, QPC], F32, tag="pq", name="qacc")
                for et in range(ET):
                    nc.tensor.matmul(
                        acc, lhsT=wq_all[:, et, qc * 128:(qc + 1) * 128],
                        rhs=xq_all[:, et, :],
                        start=(et == 0), stop=(et == ET - 1))

                def post_q(qc=qc, acc=acc):
                    vb = tmp2.tile([128, QPC], F32, tag="vb", name="qb")
                    nc.vector.tensor_scalar(
                        out=vb, in0=acc, scalar1=bq_sb[:, qc:qc + 1],
                        scalar2=None, op0=ADD)
                    sq = tmp2.tile([128, QPC], F32R, tag="sq", name="qsq")
                    nc.scalar.activation(out=sq, in_=vb, func=SQUARE)
                    ssqb = pssq2.tile([128, QPC], F32, tag="ssq", name="qssq")
                    nc.tensor.matmul(ssqb, lhsT=ones_sq_r, rhs=sq,
                                     start=True, stop=True)
                    rmsb = tmp2.tile([128, QPC], F32, tag="rms", name="qrms")
                    nc.scalar.activation(out=rmsb, in_=ssqb, func=SQRT,
                                         scale=1.0 / D, bias=eps_c[:, :])
                    rinvb = tmp2.tile([128, QPC], F32, tag="rinv",
                                      name="qrinv")
                    nc.vector.reciprocal_approx_fast(out=rinvb, in_=rmsb)
                    nc.vector.scalar_tensor_tensor(
                        out=qtn[:, qc, :], in0=vb, scalar=gq_sb[:, 0:1],
                        in1=rinvb, op0=MULT, op1=MULT)
                pending.append(post_q)
                if len(pending) > 1:
                    pending.pop(0)()
            while pending:
                pending.pop(0)()
        pwq.close()

        # ---------------- phase 3: attention + out proj -----------------
        with ExitStack() as p34:
            ctxp = p34.enter_context(tc.tile_pool(name="ctxp", bufs=1))
            ctxt = ctxp.tile([128, NH, QPC], BF16, tag="ctxt", name="ctxt")
            wop = p34.enter_context(tc.tile_pool(name="wos", bufs=1))
            wo_all = wop.tile([128, ET, E], BF16, tag="wo_all")
            for et in range(ET):
                nc.sync.dma_start(out=wo_all[:, et, :], in_=wo_r[:, et, :])
            ptp = p34.enter_context(tc.tile_pool(name="pt", bufs=3))
            hdp = p34.enter_context(tc.tile_pool(name="hdp", bufs=2))
            osb = p34.enter_context(tc.tile_pool(name="osb", bufs=2))
            oap = p34.enter_context(tc.tile_pool(name="oacc", bufs=1))
            outacc = oap.tile([128, ET, QPC], F32, tag="outacc")
            pcx = p34.enter_context(tc.tile_pool(name="pcx", bufs=2, space="PSUM"))
            pden = p34.enter_context(tc.tile_pool(name="pden", bufs=2, space="PSUM"))
            pending2 = []

            def flush2():
                while pending2:
                    pending2.pop(0)()

            def head_common(h, den, cx):
                def post_head(h=h, den=den, cx=cx):
                    rdb = hdp.tile([128, QPC], F32, tag="rd", name="rd")
                    nc.vector.reciprocal_approx_fast(out=rdb, in_=den)
                    nc.vector.tensor_tensor(out=ctxt[:, h, :], in0=cx,
                                            in1=rdb, op=MULT)
                pending2.append(post_head)

            # heads 0-7: paired-EXP pipeline (scalar would otherwise bind)
            with ExitStack() as pA:
                psc = pA.enter_context(tc.tile_pool(name="pscp", bufs=2,
                                                    space="PSUM"))
                for h in range(GS):
                    g = h // GS
                    den = pden.tile([128, QPC], F32, tag="den", name="den")
                    cx = pcx.tile([128, QPC], F32, tag="cx", name="cx")
                    for gi in range(4):
                        W = QPC - 128 * gi
                        q_ap = qtn[:, h, 128 * gi:QPC]
                        for mp in range(2):
                            sc2 = psc.tile([128, 2, QPC], F32, tag="sc",
                                           name="sc")
                            for j in range(2):
                                kt = 4 * gi + 2 * mp + j
                                nc.tensor.matmul(
                                    sc2[:, j, 0:W],
                                    lhsT=ktn[g][:, kt * 128:(kt + 1) * 128],
                                    rhs=q_ap, start=True, stop=True)

                            def post_sc(h=h, g=g, gi=gi, mp=mp, W=W,
                                        sc2=sc2, den=den, cx=cx):
                                pt2 = ptp.tile([128, 2, QPC], BF16,
                                               tag="pt", name="pt")
                                nc.scalar.activation(
                                    out=pt2[:, :, 0:W], in_=sc2[:, :, 0:W],
                                    func=EXP, scale=SCALE)
                                for j in range(2):
                                    kt = 4 * gi + 2 * mp + j
                                    nc.vector.tensor_tensor(
                                        out=pt2[:, j, 0:128],
                                        in0=pt2[:, j, 0:128],
                                        in1=mask_sb[2 * mp + j], op=MULT)
                                    nc.tensor.matmul(
                                        den[:, 128 * gi:QPC],
                                        lhsT=ones_sq_bf,
                                        rhs=pt2[:, j, 0:W],
                                        start=(kt == 0),
                                        stop=(kt == NKT - 1),
                                        skip_group_check=True)
                                    nc.tensor.matmul(
                                        cx[:, 128 * gi:QPC],
                                        lhsT=vtok[:, kt, g * D:(g + 1) * D],
                                        rhs=pt2[:, j, 0:W],
                                        start=(kt == 0),
                                        stop=(kt == NKT - 1),
                                        skip_group_check=True)
                            pending2.append(post_sc)
                            if len(pending2) > 2:
                                pending2.pop(0)()
                    head_common(h, den, cx)
                flush2()

            # heads 8-15: single-EXP pipeline, freeing two PSUM banks so the
            # ct=0..7 half of the output projection interleaves into the PE
            # bubbles (ctxt of heads 0-7 is final by now)
            with ExitStack() as pB:
                psc = pB.enter_context(tc.tile_pool(name="pscs", bufs=2,
                                                    space="PSUM"))
                pout = pB.enter_context(tc.tile_pool(name="pout", bufs=2,
                                                     space="PSUM"))
                ost = {"i": 0, "acc": None}

                def emit_out_slot():
                    i = ost["i"]
                    if i >= GS * ET:
                        return
                    ost["i"] += 1
                    c2, ct = i // GS, i % GS
                    if ct == 0:
                        ost["acc"] = pout.tile([128, QPC], F32, tag="po",
                                               name="po")
                    acc = ost["acc"]
                    nc.tensor.matmul(
                        acc, lhsT=wo_all[:, ct, c2 * 128:(c2 + 1) * 128],
                        rhs=ctxt[:, ct, :],
                        start=(ct == 0), stop=(ct == GS - 1))
                    if ct == GS - 1:
                        nc.scalar.copy(out=outacc[:, c2, :], in_=acc)

                for h in range(GS, NH):
                    g = h // GS
                    den = pden.tile([128, QPC], F32, tag="den", name="den")
                    cx = pcx.tile([128, QPC], F32, tag="cx", name="cx")
                    for gi in range(4):
                        W = QPC - 128 * gi
                        q_ap = qtn[:, h, 128 * gi:QPC]
                        for m in range(4):
                            kt = 4 * gi + m
                            sc = psc.tile([128, QPC], F32, tag="sc",
                                          name="sc")
                            nc.tensor.matmul(
                                sc[:, 0:W],
                                lhsT=ktn[g][:, kt * 128:(kt + 1) * 128],
                                rhs=q_ap, start=True, stop=True)

                            def post_sc(h=h, g=g, gi=gi, m=m, kt=kt, W=W,
                                        sc=sc, den=den, cx=cx):
                                pt = ptp.tile([128, QPC], BF16, tag="pt1",
                                              name="pt1")
                                nc.scalar.activation(
                                    out=pt[:, 0:W], in_=sc[:, 0:W],
                                    func=EXP, scale=SCALE)
                                nc.vector.tensor_tensor(
                                    out=pt[:, 0:128], in0=pt[:, 0:128],
                                    in1=mask_sb[m], op=MULT)
                                emit_out_slot()
                                nc.tensor.matmul(
                                    den[:, 128 * gi:QPC], lhsT=ones_sq_bf,
                                    rhs=pt[:, 0:W], start=(kt == 0),
                                    stop=(kt == NKT - 1),
                                    skip_group_check=True)
                                nc.tensor.matmul(
                                    cx[:, 128 * gi:QPC],
                                    lhsT=vtok[:, kt, g * D:(g + 1) * D],
                                    rhs=pt[:, 0:W], start=(kt == 0),
                                    stop=(kt == NKT - 1),
                                    skip_group_check=True)
                            pending2.append(post_sc)
                            if len(pending2) > 3:
                                pending2.pop(0)()
                    head_common(h, den, cx)
                flush2()

                # remaining half-contraction (ct=8..15) + fused combine
                for c2 in range(ET):
                    acc = pout.tile([128, QPC], F32, tag="po", name="po2")
                    for ct in range(GS, ET):
                        nc.tensor.matmul(
                            acc,
                            lhsT=wo_all[:, ct, c2 * 128:(c2 + 1) * 128],
                            rhs=ctxt[:, ct, :],
                            start=(ct == GS), stop=(ct == ET - 1))

                    def post_o(c2=c2, acc=acc):
                        ot = osb.tile([128, QPC], BF16, tag="ot", name="ot")
                        nc.vector.scalar_tensor_tensor(
                            out=ot, in0=acc, scalar=bo_sb[:, c2:c2 + 1],
                            in1=outacc[:, c2, :], op0=ADD, op1=ADD)
                        nc.sync.dma_start(
                            out=out_d[c2 * 128:(c2 + 1) * 128, :], in_=ot)
                    pending2.append(post_o)
                    if len(pending2) > 1:
                        pending2.pop(0)()
                flush2()
    nc.compile()
    return nc


# ---------------------------------------------------------------------------
# host-side sharding
# ---------------------------------------------------------------------------

def core_masks(cfg, rr):
    """[4*128, 128] bf16 diag-band masks: keep iff 128m + k <= 4q + r."""
    import ml_dtypes
    m = np.zeros((4 * 128, 128), np.float32)
    kk = np.arange(128)[:, None]
    qq = np.arange(128)[None, :]
    for t in range(4):
        m[t * 128:(t + 1) * 128, :] = (128 * t + kk <= 4 * qq + rr)
    return m.astype(ml_dtypes.bfloat16)


def make_in_maps(cfg, inputs):
    import ml_dtypes
    BF = ml_dtypes.bfloat16
    B, S, E, D, G = cfg["B"], cfg["S"], cfg["E"], cfg["D"], cfg["G"]
    NH, ET, NKT, QPC, GS = derived(cfg)
    x = np.asarray(inputs["x"], np.float32)
    shared = dict(
        Wq=np.ascontiguousarray(np.asarray(inputs["Wq"], np.float32)).astype(BF),
        Wk=np.ascontiguousarray(np.asarray(inputs["Wk"], np.float32)).astype(BF),
        Wv=np.ascontiguousarray(np.asarray(inputs["Wv"], np.float32)).astype(BF),
        Wo=np.ascontiguousarray(np.asarray(inputs["Wo"], np.float32)).astype(BF),
        bq_t=np.ascontiguousarray(
            np.asarray(inputs["bq"], np.float32).reshape(ET, 128).T),
        bk_t=np.ascontiguousarray(
            np.asarray(inputs["bk"], np.float32).reshape(G, 128).T),
        bv_r=np.asarray(inputs["bv"], np.float32).reshape(1, G * D).astype(BF),
        bo_t=np.ascontiguousarray(
            np.asarray(inputs["bo"], np.float32).reshape(ET, 128).T),
        gq_c=np.ascontiguousarray(
            np.asarray(inputs["gamma_q"], np.float32).reshape(128, 1)),
        gk_c=np.ascontiguousarray(
            np.asarray(inputs["gamma_k"], np.float32).reshape(128, 1)),
        ones_sq=np.ones((128, 128), np.float32),
    )
    xTb = [np.ascontiguousarray(x[b].T).astype(BF) for b in range(B)]
    in_maps = []
    for c in range(8):
        b, rr = c // 4, c % 4
        m = dict(shared)
        m["xT"] = xTb[b]
        m["xq"] = np.ascontiguousarray(xTb[b][:, rr::4])
        m["mask"] = core_masks(cfg, rr)
        in_maps.append(m)
    return in_maps


def assemble(cfg, results):
    B, S, E = cfg["B"], cfg["S"], cfg["E"]
    out = np.empty((B, S, E), np.float32)
    for c in range(8):
        b, rr = c // 4, c % 4
        out[b, rr::4, :] = results[c]["outT"].T.astype(np.float32)
    return out


_CACHE = {}


def kernel(**inputs):
    cfg = full_cfg()
    if "nc" not in _CACHE:
        _CACHE["nc"] = build_program(cfg)
    nc = _CACHE["nc"]
    in_maps = make_in_maps(cfg, inputs)
    res = run_bass_kernel_spmd(nc, in_maps, list(range(8)))
    return assemble(cfg, res.results)


# revision 9
# speedup vs baseline: 1.0329x; 1.0124x over previous
"""GQA attention block (RMSNorm-QK, causal, GQA) on 8 trn2 NeuronCores.

v13: strided-query sharding, bf16 end-to-end (bf16 output, host-widened),
causal skip, zero collectives, demand-ordered DMA issue, and the output
projection's first half-contraction interleaved into the attention
pipeline.

Core c handles batch c//4 and query tokens {t : t % 4 == r}, r = c%4. Strided
queries make the causal structure identical on every core (token of local
query i is 4i+r), so one uniform SPMD program can *skip* fully-masked key
tiles: scores/den/context matmuls for key-tile kt only cover query columns
[128*(kt//4) : 512] — 5120 moving columns per head instead of 8192. The
128-token diagonal band is handled by 4 per-core [128,128] mask tiles
(host data), multiplied into the first 128 columns post-exp. Every core
projects K/V for its full batch locally (a 4-way-sharded AllGather variant
had the same mean latency with far higher variance). DMA issues are
demand-ordered: all x chunks, then xq, then Wq just-in-time for Q-proj.
Heads 0-7 run a paired-EXP attention pipeline (scalar would otherwise
bind); heads 8-15 drop pairing to free two PSUM banks so the ct=0..7 half
of the output projection streams into the attention PE bubbles (fillers
emitted before the dependency-stalled matmuls — the PE queue is in-order),
halving the serial out-proj tail; the recombine fuses (psum + bias) +
partial in one scalar_tensor_tensor.

All big matmuls are bf16 (host-converted weights/x): full PE rate, 4x faster
weight loads, half the DMA. Rank-1 broadcast matmuls (RMS/softmax denom
broadcasts) are f32r (fp32 runs at 1/4 rate). Softmax denominators use
reciprocal_approx_fast (5x faster than reciprocal; 18-bit accuracy is far
below the 2e-2 gate). V is projected directly in [token, feature] layout
(x-tile stationary, Wv moving) so no PE transposes exist anywhere. Wq/Wo
live whole in SBUF (8MB bf16 each), chunk-DMA'd during the preceding phase.
"""

import math
import numpy as np
from contextlib import ExitStack

import concourse.bass as bass
import concourse.mybir as mybir
import concourse.tile as tile
from concourse import bacc
from concourse.bass_utils import run_bass_kernel_spmd

F32 = mybir.dt.float32
F32R = mybir.dt.float32r
BF16 = mybir.dt.bfloat16
ADD = mybir.AluOpType.add
MULT = mybir.AluOpType.mult
EXP = mybir.ActivationFunctionType.Exp
SQRT = mybir.ActivationFunctionType.Sqrt
SQUARE = mybir.ActivationFunctionType.Square

EPS = 1e-8


def full_cfg():
    return dict(B=2, S=2048, E=2048, D=128, G=2)


def derived(cfg):
    B, S, E, D, G = cfg["B"], cfg["S"], cfg["E"], cfg["D"], cfg["G"]
    NH = E // D            # 16 query heads
    ET = E // 128          # 16 feature tiles
    NKT = S // 128         # 16 key tiles
    QPC = S // 4           # 512 queries per core
    GS = NH // G           # 8 heads per kv group
    assert D == 128 and QPC == 512
    return NH, ET, NKT, QPC, GS


def build_program(cfg):
    B, S, E, D, G = cfg["B"], cfg["S"], cfg["E"], cfg["D"], cfg["G"]
    NH, ET, NKT, QPC, GS = derived(cfg)
    SCALE = 1.0 / math.sqrt(D)
    KC = 512               # token-chunk width for K/V projection
    NKC = S // KC
    GD = G * D             # 256

    nc = bacc.Bacc()
    xT_d = nc.dram_tensor("xT", [E, S], BF16, kind="ExternalInput")
    xq_d = nc.dram_tensor("xq", [E, QPC], BF16, kind="ExternalInput")
    wq_d = nc.dram_tensor("Wq", [E, E], BF16, kind="ExternalInput")
    wk_d = nc.dram_tensor("Wk", [E, GD], BF16, kind="ExternalInput")
    wv_d = nc.dram_tensor("Wv", [E, GD], BF16, kind="ExternalInput")
    wo_d = nc.dram_tensor("Wo", [E, E], BF16, kind="ExternalInput")
    bq_d = nc.dram_tensor("bq_t", [128, ET], F32, kind="ExternalInput")
    bk_d = nc.dram_tensor("bk_t", [128, G], F32, kind="ExternalInput")
    bv_d = nc.dram_tensor("bv_r", [1, GD], BF16, kind="ExternalInput")
    bo_d = nc.dram_tensor("bo_t", [128, ET], F32, kind="ExternalInput")
    gq_d = nc.dram_tensor("gq_c", [128, 1], F32, kind="ExternalInput")
    gk_d = nc.dram_tensor("gk_c", [128, 1], F32, kind="ExternalInput")
    mask_d = nc.dram_tensor("mask", [4 * 128, 128], BF16, kind="ExternalInput")
    onesq_d = nc.dram_tensor("ones_sq", [128, 128], F32, kind="ExternalInput")
    out_d = nc.dram_tensor("outT", [E, QPC], BF16, kind="ExternalOutput")

    wq_r = wq_d.rearrange("(t p) c -> p t c", p=128)   # [128, ET, E]
    wk_r = wk_d.rearrange("(t p) c -> p t c", p=128)   # [128, ET, GD]
    wv_r = wv_d.rearrange("(t p) c -> p t c", p=128)
    wo_r = wo_d.rearrange("(t p) c -> p t c", p=128)

    def r(ap):
        return ap if ap.dtype == F32R else ap.bitcast(F32R)

    with tile.TileContext(nc) as tc, ExitStack() as top:
        consts = top.enter_context(tc.tile_pool(name="consts", bufs=1))
        persist = top.enter_context(tc.tile_pool(name="persist", bufs=1))

        ones_sq_r = consts.tile([128, 128], F32R)
        ones_sq_bf = consts.tile([128, 128], BF16)
        nc.vector.memset(ones_sq_bf, 1.0)
        onesrow_bf = consts.tile([1, 128], BF16)
        nc.vector.memset(onesrow_bf, 1.0)
        eps_c = consts.tile([128, 1], F32)
        nc.vector.memset(eps_c, EPS)
        gq_sb = consts.tile([128, 1], F32)
        gk_sb = consts.tile([128, 1], F32)
        bq_sb = consts.tile([128, ET], F32)
        bk_sb = consts.tile([128, G], F32)
        bv_sb = consts.tile([1, GD], BF16)
        bo_sb = consts.tile([128, ET], F32)
        mask_sb = [consts.tile([128, 128], BF16, tag=f"mask{m}",
                               name=f"mask{m}") for m in range(4)]

        def issue_const_dmas():
            nc.sync.dma_start(out=ones_sq_r, in_=onesq_d[:, :].bitcast(F32R))
            nc.sync.dma_start(out=gq_sb, in_=gq_d[:, :])
            nc.sync.dma_start(out=gk_sb, in_=gk_d[:, :])
            nc.sync.dma_start(out=bq_sb, in_=bq_d[:, :])
            nc.sync.dma_start(out=bk_sb, in_=bk_d[:, :])
            nc.sync.dma_start(out=bv_sb, in_=bv_d[:, :])
            nc.sync.dma_start(out=bo_sb, in_=bo_d[:, :])
            for m in range(4):
                nc.sync.dma_start(out=mask_sb[m],
                                  in_=mask_d[m * 128:(m + 1) * 128, :])

        ktn = [persist.tile([128, S], BF16, tag=f"ktn{g}", name=f"ktn{g}")
               for g in range(G)]
        vtok = persist.tile([128, NKT, GD], BF16, tag="vtok")
        qtn = persist.tile([128, NH, QPC], BF16, tag="qtn")

        # ---------------- phase 1: K/V projections ----------------------
        # wq_all/xq_all persist into phase 2 (freed before Wo loads); DMAs
        # are issued mid-phase-1 so the transfers hide under K/V compute.
        pwq = ExitStack()
        wqxq = pwq.enter_context(tc.tile_pool(name="wqxq", bufs=1))
        wq_all = wqxq.tile([128, ET, E], BF16, tag="wq_all")
        xq_all = wqxq.tile([128, ET, QPC], BF16, tag="xq_all")

        with ExitStack() as p1:
            wkvp = p1.enter_context(tc.tile_pool(name="wkv", bufs=1))
            xsp = p1.enter_context(tc.tile_pool(name="xs", bufs=20))
            tmp = p1.enter_context(tc.tile_pool(name="tmp1", bufs=3))
            pkv = p1.enter_context(tc.tile_pool(name="pkv", bufs=3, space="PSUM"))
            pv = p1.enter_context(tc.tile_pool(name="pv", bufs=2, space="PSUM"))
            pssq = p1.enter_context(tc.tile_pool(name="pssq", bufs=2, space="PSUM"))

            wk_sb = wkvp.tile([128, ET, GD], BF16, tag="wk")
            wv_sb = wkvp.tile([128, ET, GD], BF16, tag="wv")

            pending = []

            def flush():
                while pending:
                    pending.pop(0)()

            for kc in range(NKC):
                xts = []
                for et in range(ET):
                    xt = xsp.tile([128, KC], BF16, tag="xt")
                    nc.sync.dma_start(
                        out=xt, in_=xT_d[et * 128:(et + 1) * 128,
                                         kc * KC:(kc + 1) * KC])
                    xts.append(xt)
                    if kc == 0:
                        # interleave weight chunks across DMA queues so the
                        # first matmul isn't gated on a single 1MB transfer
                        nc.sync.dma_start(out=wk_sb[:, et, :],
                                          in_=wk_r[:, et, :])
                        nc.sync.dma_start(out=wv_sb[:, et, :],
                                          in_=wv_r[:, et, :])
                if kc == 0:
                    issue_const_dmas()
                if kc == NKC - 1:
                    # demand-order the queues: all x chunks land first (the
                    # K/V loop needs kc3 by ~49us), then xq, then Wq whose
                    # first use is Q-proj at ~62us — issuing Wq earlier put
                    # it AHEAD of kc2/kc3 x-tiles in the DMA queues and
                    # starved the K/V loop
                    for et in range(ET):
                        nc.sync.dma_start(
                            out=xq_all[:, et, :],
                            in_=xq_d[et * 128:(et + 1) * 128, :])
                    for et in range(ET):
                        nc.sync.dma_start(out=wq_all[:, et, :],
                                          in_=wq_r[:, et, :])

                # K projection: feature-major [d, token]
                for g in range(G):
                    acc = pkv.tile([128, KC], F32, tag="pkv", name="kacc")
                    for et in range(ET):
                        nc.tensor.matmul(
                            acc, lhsT=wk_sb[:, et, g * D:(g + 1) * D],
                            rhs=xts[et], start=(et == 0), stop=(et == ET - 1))

                    def post_k(kc=kc, g=g, acc=acc):
                        vb = tmp.tile([128, KC], F32, tag="vb", name="kb")
                        nc.vector.tensor_scalar(
                            out=vb, in0=acc, scalar1=bk_sb[:, g:g + 1],
                            scalar2=None, op0=ADD)
                        sq = tmp.tile([128, KC], F32R, tag="sq", name="ksq")
                        nc.scalar.activation(out=sq, in_=vb, func=SQUARE)
                        # sum over partitions, pre-broadcast to all 128 rows
                        ssqb = pssq.tile([128, KC], F32, tag="ssq",
                                         name="kssq")
                        nc.tensor.matmul(ssqb, lhsT=ones_sq_r, rhs=sq,
                                         start=True, stop=True)
                        rmsb = tmp.tile([128, KC], F32, tag="rms",
                                        name="krms")
                        nc.scalar.activation(out=rmsb, in_=ssqb, func=SQRT,
                                             scale=1.0 / D, bias=eps_c[:, :])
                        rinvb = tmp.tile([128, KC], F32, tag="rinv",
                                         name="krinv")
                        nc.vector.reciprocal_approx_fast(out=rinvb, in_=rmsb)
                        nc.vector.scalar_tensor_tensor(
                            out=ktn[g][:, kc * KC:(kc + 1) * KC],
                            in0=vb, scalar=gk_sb[:, 0:1], in1=rinvb,
                            op0=MULT, op1=MULT)
                    pending.append(post_k)

                # V projection directly as [token, feature(GD)]
                for s2 in range(2):
                    vt2 = pv.tile([128, 2, GD], F32, tag="pv", name="vt2")
                    for s in range(2):
                        sub = 2 * s2 + s
                        for et in range(ET):
                            nc.tensor.matmul(
                                vt2[:, s, :],
                                lhsT=xts[et][:, sub * 128:(sub + 1) * 128],
                                rhs=wv_sb[:, et, :],
                                start=(et == 0), stop=False)
                        nc.tensor.matmul(
                            vt2[:, s, :], lhsT=onesrow_bf, rhs=bv_sb,
                            start=False, stop=True)

                    def post_v(kc=kc, s2=s2, vt2=vt2):
                        kt0 = kc * 4 + s2 * 2
                        nc.scalar.copy(out=vtok[:, kt0:kt0 + 2, :], in_=vt2)
                    pending.append(post_v)
                    if len(pending) > 2:
                        pending.pop(0)()
            flush()

        # ---------------- phase 2: Q projection -------------------------
        with ExitStack() as p2:
            tmp2 = p2.enter_context(tc.tile_pool(name="tmp2", bufs=3))
            pq = p2.enter_context(tc.tile_pool(name="pq", bufs=2, space="PSUM"))
            pssq2 = p2.enter_context(tc.tile_pool(name="pssq2", bufs=2, space="PSUM"))
            pending = []
            for qc in range(NH):
                acc = pq.tile([128, QPC], F32, tag="pq", name="qacc")
                for et in range(ET):
                    nc.tensor.matmul(
                        acc, lhsT=wq_all[:, et, qc * 128:(qc + 1) * 128],
                        rhs=xq_all[:, et, :],
                        start=(et == 0), stop=(et == ET - 1))

                def post_q(qc=qc, acc=acc):
                    vb = tmp2.tile([128, QPC], F32, tag="vb", name="qb")
                    nc.vector.tensor_scalar(
                        out=vb, in0=acc, scalar1=bq_sb[:, qc:qc + 1],
                        scalar2=None, op0=ADD)
                    sq = tmp2.tile([128, QPC], F32R, tag="sq", name="qsq")
                    nc.scalar.activation(out=sq, in_=vb, func=SQUARE)
                    ssqb = pssq2.tile([128, QPC], F32, tag="ssq", name="qssq")
                    nc.tensor.matmul(ssqb, lhsT=ones_sq_r, rhs=sq,
                                     start=True, stop=True)
                    rmsb = tmp2.tile([128, QPC], F32, tag="rms", name="qrms")
                    nc.scalar.activation(out=rmsb, in_=ssqb, func=SQRT,
                                         scale=1.0 / D, bias=eps_c[:, :])
                    rinvb = tmp2.tile([128, QPC], F32, tag="rinv",
                                      name="qrinv")
                    nc.vector.reciprocal_approx_fast(out=rinvb, in_=rmsb)
                    nc.vector.scalar_tensor_tensor(
                        out=qtn[:, qc, :], in0=vb, scalar=gq_sb[:, 0:1],
                        in1=rinvb, op0=MULT, op1=MULT)
                pending.append(post_q)
                if len(pending) > 1:
                    pending.pop(0)()
            while pending:
                pending.pop(0)()
        pwq.close()

        # ---------------- phase 3: attention + out proj -----------------
        with ExitStack() as p34:
            ctxp = p34.enter_context(tc.tile_pool(name="ctxp", bufs=1))
            ctxt = ctxp.tile([128, NH, QPC], BF16, tag="ctxt", name="ctxt")
            wop = p34.enter_context(tc.tile_pool(name="wos", bufs=1))
            wo_all = wop.tile([128, ET, E], BF16, tag="wo_all")
            for et in range(ET):
                nc.sync.dma_start(out=wo_all[:, et, :], in_=wo_r[:, et, :])
            ptp = p34.enter_context(tc.tile_pool(name="pt", bufs=3))
            hdp = p34.enter_context(tc.tile_pool(name="hdp", bufs=2))
            osb = p34.enter_context(tc.tile_pool(name="osb", bufs=2))
            oap = p34.enter_context(tc.tile_pool(name="oacc", bufs=1))
            outacc = oap.tile([128, ET, QPC], F32, tag="outacc")
            pcx = p34.enter_context(tc.tile_pool(name="pcx", bufs=2, space="PSUM"))
            pden = p34.enter_context(tc.tile_pool(name="pden", bufs=2, space="PSUM"))
            pending2 = []

            def flush2():
                while pending2:
                    pending2.pop(0)()

            def head_common(h, den, cx):
                def post_head(h=h, den=den, cx=cx):
                    rdb = hdp.tile([128, QPC], F32, tag="rd", name="rd")
                    nc.vector.reciprocal_approx_fast(out=rdb, in_=den)
                    nc.vector.tensor_tensor(out=ctxt[:, h, :], in0=cx,
                                            in1=rdb, op=MULT)
                pending2.append(post_head)

            # heads 0-7: paired-EXP pipeline (scalar would otherwise bind)
            with ExitStack() as pA:
                psc = pA.enter_context(tc.tile_pool(name="pscp", bufs=2,
                                                    space="PSUM"))
                for h in range(GS):
                    g = h // GS
                    den = pden.tile([128, QPC], F32, tag="den", name="den")
                    cx = pcx.tile([128, QPC], F32, tag="cx", name="cx")
                    for gi in range(4):
                        W = QPC - 128 * gi
                        q_ap = qtn[:, h, 128 * gi:QPC]
                        for mp in range(2):
                            sc2 = psc.tile([128, 2, QPC], F32, tag="sc",
                                           name="sc")
                            for j in range(2):
                                kt = 4 * gi + 2 * mp + j
                                nc.tensor.matmul(
                                    sc2[:, j, 0:W],
                                    lhsT=ktn[g][:, kt * 128:(kt + 1) * 128],
                                    rhs=q_ap, start=True, stop=True)

                            def post_sc(h=h, g=g, gi=gi, mp=mp, W=W,
                                        sc2=sc2, den=den, cx=cx):
                                pt2 = ptp.tile([128, 2, QPC], BF16,
                                               tag="pt", name="pt")
                                nc.scalar.activation(
                                    out=pt2[:, :, 0:W], in_=sc2[:, :, 0:W],
                                    func=EXP, scale=SCALE)
                                for j in range(2):
                                    kt = 4 * gi + 2 * mp + j
                                    nc.vector.tensor_tensor(
                                        out=pt2[:, j, 0:128],
                                        in0=pt2[:, j, 0:128],
                                        in1=mask_sb[2 * mp + j], op=MULT)
                                    nc.tensor.matmul(
                                        den[:, 128 * gi:QPC],
                                        lhsT=ones_sq_bf,
                                        rhs=pt2[:, j, 0:W],
                                        start=(kt == 0),
                                        stop=(kt == NKT - 1),
                                        skip_group_check=True)
                                    nc.tensor.matmul(
                                        cx[:, 128 * gi:QPC],
                                        lhsT=vtok[:, kt, g * D:(g + 1) * D],
                                        rhs=pt2[:, j, 0:W],
                                        start=(kt == 0),
                                        stop=(kt == NKT - 1),
                                        skip_group_check=True)
                            pending2.append(post_sc)
                            if len(pending2) > 2:
                                pending2.pop(0)()
                    head_common(h, den, cx)
                flush2()

            # heads 8-15: single-EXP pipeline, freeing two PSUM banks so the
            # ct=0..7 half of the output projection interleaves into the PE
            # bubbles (ctxt of heads 0-7 is final by now)
            with ExitStack() as pB:
                psc = pB.enter_context(tc.tile_pool(name="pscs", bufs=2,
                                                    space="PSUM"))
                pout = pB.enter_context(tc.tile_pool(name="pout", bufs=2,
                                                     space="PSUM"))
                ost = {"i": 0, "acc": None}

                def emit_out_slot():
                    i = ost["i"]
                    if i >= GS * ET:
                        return
                    ost["i"] += 1
                    c2, ct = i // GS, i % GS
                    if ct == 0:
                        ost["acc"] = pout.tile([128, QPC], F32, tag="po",
                                               name="po")
                    acc = ost["acc"]
                    nc.tensor.matmul(
                        acc, lhsT=wo_all[:, ct, c2 * 128:(c2 + 1) * 128],
                        rhs=ctxt[:, ct, :],
                        start=(ct == 0), stop=(ct == GS - 1))
                    if ct == GS - 1:
                        nc.scalar.copy(out=outacc[:, c2, :], in_=acc)

                for h in range(GS, NH):
                    g = h // GS
                    den = pden.tile([128, QPC], F32, tag="den", name="den")
                    cx = pcx.tile([128, QPC], F32, tag="cx", name="cx")
                    for gi in range(4):
                        W = QPC - 128 * gi
                        q_ap = qtn[:, h, 128 * gi:QPC]
                        for m in range(4):
                            kt = 4 * gi + m
                            sc = psc.tile([128, QPC], F32, tag="sc",
                                          name="sc")
                            nc.tensor.matmul(
                                sc[:, 0:W],
                                lhsT=ktn[g][:, kt * 128:(kt + 1) * 128],
                                rhs=q_ap, start=True, stop=True)

                            def post_sc(h=h, g=g, gi=gi, m=m, kt=kt, W=W,
                                        sc=sc, den=den, cx=cx):
                                pt = ptp.tile([128, QPC], BF16, tag="pt1",
                                              name="pt1")
                                nc.scalar.activation(
                                    out=pt[:, 0:W], in_=sc[:, 0:W],
                                    func=EXP, scale=SCALE)
                                nc.vector.tensor_tensor(
                                    out=pt[:, 0:128], in0=pt[:, 0:128],
                                    in1=mask_sb[m], op=MULT)
                                emit_out_slot()
                                nc.tensor.matmul(
                                    den[:, 128 * gi:QPC], lhsT=ones_sq_bf,
                                    rhs=pt[:, 0:W], start=(kt == 0),
                                    stop=(kt == NKT - 1),
                                    skip_group_check=True)
                                nc.tensor.matmul(
                                    cx[:, 128 * gi:QPC],
                                    lhsT=vtok[:, kt, g * D:(g + 1) * D],
                                    rhs=pt[:, 0:W], start=(kt == 0),
                                    stop=(kt == NKT - 1),
                                    skip_group_check=True)
                            pending2.append(post_sc)
                            if len(pending2) > 3:
                                pending2.pop(0)()
                    head_common(h, den, cx)
                flush2()

                # remaining half-contraction (ct=8..15) + fused combine
                for c2 in range(ET):
                    acc = pout.tile([128, QPC], F32, tag="po", name="po2")
                    for ct in range(GS, ET):
                        nc.tensor.matmul(
                            acc,
                            lhsT=wo_all[:, ct, c2 * 128:(c2 + 1) * 128],
                            rhs=ctxt[:, ct, :],
                            start=(ct == GS), stop=(ct == ET - 1))

                    def post_o(c2=c2, acc=acc):
                        ot = osb.tile([128, QPC], BF16, tag="ot", name="ot")
                        nc.vector.scalar_tensor_tensor(
                            out=ot, in0=acc, scalar=bo_sb[:, c2:c2 + 1],
                            in1=outacc[:, c2, :], op0=ADD, op1=ADD)
                        nc.sync.dma_start(
                            out=out_d[c2 * 128:(c2 + 1) * 128, :], in_=ot)
                    pending2.append(post_o)
                    if len(pending2) > 1:
                        pending2.pop(0)()
                flush2()
    nc.compile()
    return nc


# ---------------------------------------------------------------------------
# host-side sharding
# ---------------------------------------------------------------------------

def core_masks(cfg, rr):
    """[4*128, 128] bf16 diag-band masks: keep iff 128m + k <= 4q + r."""
    import ml_dtypes
    m = np.zeros((4 * 128, 128), np.float32)
    kk = np.arange(128)[:, None]
    qq = np.arange(128)[None, :]
    for t in range(4):
        m[t * 128:(t + 1) * 128, :] = (128 * t + kk <= 4 * qq + rr)
    return m.astype(ml_dtypes.bfloat16)


def make_in_maps(cfg, inputs):
    import ml_dtypes
    BF = ml_dtypes.bfloat16
    B, S, E, D, G = cfg["B"], cfg["S"], cfg["E"], cfg["D"], cfg["G"]
    NH, ET, NKT, QPC, GS = derived(cfg)
    x = np.asarray(inputs["x"], np.float32)
    shared = dict(
        Wq=np.ascontiguousarray(np.asarray(inputs["Wq"], np.float32)).astype(BF),
        Wk=np.ascontiguousarray(np.asarray(inputs["Wk"], np.float32)).astype(BF),
        Wv=np.ascontiguousarray(np.asarray(inputs["Wv"], np.float32)).astype(BF),
        Wo=np.ascontiguousarray(np.asarray(inputs["Wo"], np.float32)).astype(BF),
        bq_t=np.ascontiguousarray(
            np.asarray(inputs["bq"], np.float32).reshape(ET, 128).T),
        bk_t=np.ascontiguousarray(
            np.asarray(inputs["bk"], np.float32).reshape(G, 128).T),
        bv_r=np.asarray(inputs["bv"], np.float32).reshape(1, G * D).astype(BF),
        bo_t=np.ascontiguousarray(
            np.asarray(inputs["bo"], np.float32).reshape(ET, 128).T),
        gq_c=np.ascontiguousarray(
            np.asarray(inputs["gamma_q"], np.float32).reshape(128, 1)),
        gk_c=np.ascontiguousarray(
            np.asarray(inputs["gamma_k"], np.float32).reshape(128, 1)),
        ones_sq=np.ones((128, 128), np.float32),
    )
    xTb = [np.ascontiguousarray(x[b].T).astype(BF) for b in range(B)]
    in_maps = []
    for c in range(8):
        b, rr = c // 4, c % 4
        m = dict(shared)
        m["xT"] = xTb[b]
        m["xq"] = np.ascontiguousarray(xTb[b][:, rr::4])
        m["mask"] = core_masks(cfg, rr)
        in_maps.append(m)
    return in_maps


def assemble(cfg, results):
    B, S, E = cfg["B"], cfg["S"], cfg["E"]
    out = np.empty((B, S, E), np.float32)
    for c in range(8):
        b, rr = c // 4, c % 4
        out[b, rr::4, :] = results[c]["outT"].T.astype(np.float32)
    return out


_CACHE = {}


def kernel(**inputs):
    cfg = full_cfg()
    if "nc" not in _CACHE:
        _CACHE["nc"] = build_program(cfg)
    nc = _CACHE["nc"]
    in_maps = make_in_maps(cfg, inputs)
    res = run_bass_kernel_spmd(nc, in_maps, list(range(8)))
    return assemble(cfg, res.results)
